# revision 46
# baseline (speedup 1.0000x reference)
"""Trainium2 Bass kernel for nn_Distance2logprob (retrieval_knn).

out[n,m] = keep ? -d[n,m] - log(Z[n]) : -inf
  d[n,m] = ||e_n - r_m||^2,  Z[n] = sum_m keep[n,m]*exp(-d[n,m]),
  keep = (inputs == 0)

Strategy (8 NeuronCores, data-parallel over N; ref_weight replicated):
  factor exp(-d) = exp(2*cross - sq_e) * exp(-sq_r), and fold the mask into
  the host-precomputed W[n,m] = keep * exp(-sq_r[m]) (f32, same bytes as the
  int32 mask it replaces). Per 512-col tile:
    - one bf16 matmul: psum = 2*cross        (TensorE)
    - e' = Exp(psum - sq_e)                  (ScalarE, per-partition bias)
    - emask = e' * W, Z-partial = row-sum    (VectorE tensor_tensor_reduce)
  epilogue: out = Ln(emask * (1/Z)) = -d - logZ, and Ln(0) = -inf at masked.
  The bf16 codebook halves its HBM traffic; precision impact ~1e-5 rel.
"""

import sys
from contextlib import ExitStack

import numpy as np

for _p in ("/opt/trn_rl_repo",):
    if _p not in sys.path:
        sys.path.insert(0, _p)

import concourse.bacc as bacc
import concourse.bass as bass
import concourse.tile as tile
from concourse import mybir
from concourse.bass_utils import run_bass_kernel_spmd

N, M, D = 1024, 32768, 128
NCORES = 8
NSH = N // NCORES  # 128 rows per core
CH = 2048          # DMA chunk (columns of the output)
SUB = 512          # matmul free-dim tile (one PSUM bank)
F32 = mybir.dt.float32
F32R = mybir.dt.float32r
BF16 = mybir.dt.bfloat16


def _patch_act_tables():
    """Restrict activation-table choice to the set containing BOTH exp and ln
    (id 6, natural_log_exp_and_others), so ScalarE loads its LUT once instead
    of swapping tables between every Exp and Ln instruction (~1.3 us each)."""
    import functools

    import concourse.hw_specs as hw_specs

    if getattr(bacc, "_act_tables_patched", False):
        return
    orig = hw_specs.get_activation_tables

    @functools.cache
    def only_combined(arch):
        return {
            name: (funcs if name == "natural_log_exp_and_others" else set())
            for name, funcs in orig(arch).items()
        }

    bacc.get_activation_tables = only_combined
    bacc._act_tables_patched = True


def build_bass(m_total: int = M, ch: int = CH) -> bass.Bass:
    _patch_act_tables()
    nch = m_total // ch
    nspc = ch // SUB
    nsub = m_total // SUB

    nc = bacc.Bacc(trn_type="TRN2", target_bir_lowering=False, debug=False)

    et2_d = nc.dram_tensor("et2", [D, NSH], BF16, kind="ExternalInput").ap()
    nsq_d = nc.dram_tensor("negsqe", [NSH, 1], F32, kind="ExternalInput").ap()
    refwt_d = nc.dram_tensor("refwt", [D, m_total], BF16, kind="ExternalInput").ap()
    w_d = nc.dram_tensor("wmask", [NSH, m_total], mybir.dt.float16, kind="ExternalInput").ap()
    out_d = nc.dram_tensor("out", [NSH, m_total], F32, kind="ExternalOutput").ap()

    with tile.TileContext(nc) as tc, ExitStack() as ctx:
        const = ctx.enter_context(tc.tile_pool(name="const", bufs=1))
        bigp = ctx.enter_context(tc.tile_pool(name="emaskp", bufs=1))
        rtp = ctx.enter_context(tc.tile_pool(name="rt", bufs=4))
        wp = ctx.enter_context(tc.tile_pool(name="wp", bufs=4))
        ep = ctx.enter_context(tc.tile_pool(name="ep", bufs=4))
        psp = ctx.enter_context(tc.tile_pool(name="ps", bufs=4, space="PSUM"))

        zbias = const.tile([NSH, 1], F32)
        nc.vector.memset(zbias, 0.0)
        et2 = const.tile([D, NSH], BF16)
        nc.sync.dma_start(out=et2, in_=et2_d)
        nsq = const.tile([NSH, 1], F32)
        nc.sync.dma_start(out=nsq, in_=nsq_d)

        zparts = const.tile([NSH, nsub], F32)
        emask = bigp.tile([NSH, m_total], F32)

        def emit_pre_ln(j):
            # pre-barrier Ln: emask <- ln(e_masked) = -d (-inf at masked)
            jsl = slice(j * ch, (j + 1) * ch)
            nc.scalar.activation(
                out=emask[:, jsl], in_=emask[:, jsl],
                func=mybir.ActivationFunctionType.Ln,
                bias=zbias, scale=1.0,
            )

        LAG = 2  # emit chunk j's Ln two chunks late so exps never queue behind it
        for i in range(nch):
            csl = slice(i * ch, (i + 1) * ch)
            rt = rtp.tile([D, ch], BF16)
            nc.sync.dma_start(out=rt, in_=refwt_d[:, csl])
            w = wp.tile([NSH, ch], mybir.dt.float16)
            nc.sync.dma_start(out=w, in_=w_d[:, csl])

            for s in range(nspc):
                ssl = slice(s * SUB, (s + 1) * SUB)
                k = i * nspc + s
                ksl = slice(k * SUB, (k + 1) * SUB)
                ps = psp.tile([NSH, SUB], F32)
                nc.tensor.matmul(ps, lhsT=et2, rhs=rt[:, ssl], start=True, stop=True)
                ex = ep.tile([NSH, SUB], F32)
                # e' = exp(2*cross - sq_e)
                nc.scalar.activation(
                    out=ex, in_=ps,
                    func=mybir.ActivationFunctionType.Exp,
                    bias=nsq, scale=1.0,
                )
                # emask = e' * W  (0 at masked), Z-partial = fused row-sum
                nc.vector.scalar_tensor_tensor(
                    out=emask[:, ksl], in0=ex, scalar=1.0, in1=w[:, ssl],
                    op0=mybir.AluOpType.mult, op1=mybir.AluOpType.mult,
                    accum_out=zparts[:, k:k + 1],
                )


        zsum = const.tile([NSH, 1], F32)
        nc.vector.tensor_reduce(
            zsum, zparts, axis=mybir.AxisListType.X, op=mybir.AluOpType.add
        )
        rz = const.tile([NSH, 1], F32)
        nc.vector.reciprocal(rz, zsum)
        neg_lnz = const.tile([NSH, 1], F32)
        # ln(1/Z) = -lnZ
        nc.scalar.activation(
            out=neg_lnz, in_=rz, func=mybir.ActivationFunctionType.Ln,
            bias=zbias, scale=1.0,
        )

        for i in range(nch):
            sl = slice(i * ch, (i + 1) * ch)
            # out = ln(e_masked / Z); ln(0) = -inf at masked
            nc.scalar.activation(
                out=emask[:, sl], in_=emask[:, sl],
                func=mybir.ActivationFunctionType.Ln,
                bias=zbias, scale=rz,
            )
            nc.sync.dma_start(out=out_d[:, sl], in_=emask[:, sl])

    nc.compile()
    return nc


def make_in_maps(embeddings: np.ndarray, ref_weight: np.ndarray, inputs: np.ndarray):
    import ml_dtypes

    embeddings = np.ascontiguousarray(np.asarray(embeddings, dtype=np.float32))
    ref_weight = np.asarray(ref_weight, dtype=np.float32)
    inputs = np.asarray(inputs)
    refwt = np.ascontiguousarray(ref_weight.T).astype(ml_dtypes.bfloat16)  # [D, M]
    sqr = (ref_weight * ref_weight).sum(axis=1)[None, :]                   # [1, M]
    wmask = np.where(inputs == 0, np.exp(-sqr), np.float32(0.0)).astype(np.float16)
    in_maps = []
    for c in range(NCORES):
        e = embeddings[c * NSH:(c + 1) * NSH]
        in_maps.append({
            "et2": (np.ascontiguousarray(e.T) * np.float32(2.0)).astype(ml_dtypes.bfloat16),
            "negsqe": -(e * e).sum(axis=1, keepdims=True).astype(np.float32),
            "refwt": refwt,
            "wmask": np.ascontiguousarray(wmask[c * NSH:(c + 1) * NSH]),
        })
    return in_maps


_NC_CACHE: dict = {}


def get_nc() -> bass.Bass:
    if "nc" not in _NC_CACHE:
        _NC_CACHE["nc"] = build_bass()
    return _NC_CACHE["nc"]


def kernel(embeddings: np.ndarray, ref_weight: np.ndarray, inputs: np.ndarray,
           **_ignored) -> np.ndarray:
    nc = get_nc()
    in_maps = make_in_maps(embeddings, ref_weight, inputs)
    res = run_bass_kernel_spmd(nc, in_maps, list(range(NCORES)))
    out = np.concatenate([res.results[c]["out"] for c in range(NCORES)], axis=0)
    return out


# revision 48
# speedup vs baseline: 1.0653x; 1.0653x over previous
"""Trainium2 Bass kernel for nn_Distance2logprob (retrieval_knn).

out[n,m] = keep ? -d[n,m] - log(Z[n]) : -inf
  d[n,m] = ||e_n - r_m||^2,  Z[n] = sum_m keep[n,m]*exp(-d[n,m]),
  keep = (inputs == 0)

Strategy (8 NeuronCores, data-parallel over N; ref_weight replicated):
  factor exp(-d) = exp(2*cross - sq_e) * exp(-sq_r), and fold the mask into
  the host-precomputed W[n,m] = keep * exp(-sq_r[m]) (f32, same bytes as the
  int32 mask it replaces). Per 512-col tile:
    - one bf16 matmul: psum = 2*cross        (TensorE)
    - e' = Exp(psum - sq_e)                  (ScalarE, per-partition bias)
    - emask = e' * W, Z-partial = row-sum    (VectorE tensor_tensor_reduce)
  epilogue: out = Ln(emask * (1/Z)) = -d - logZ, and Ln(0) = -inf at masked.
  The bf16 codebook halves its HBM traffic; precision impact ~1e-5 rel.
"""

import sys
from contextlib import ExitStack

import numpy as np

for _p in ("/opt/trn_rl_repo",):
    if _p not in sys.path:
        sys.path.insert(0, _p)

import concourse.bacc as bacc
import concourse.bass as bass
import concourse.tile as tile
from concourse import mybir
from concourse.bass_utils import run_bass_kernel_spmd

N, M, D = 1024, 32768, 128
NCORES = 8
NSH = N // NCORES  # 128 rows per core
CH = 2048          # DMA chunk (columns of the output)
SUB = 512          # matmul free-dim tile (one PSUM bank)
F32 = mybir.dt.float32
F32R = mybir.dt.float32r
BF16 = mybir.dt.bfloat16


def _patch_act_tables():
    """Restrict activation-table choice to the set containing BOTH exp and ln
    (id 6, natural_log_exp_and_others), so ScalarE loads its LUT once instead
    of swapping tables between every Exp and Ln instruction (~1.3 us each)."""
    import functools

    import concourse.hw_specs as hw_specs

    if getattr(bacc, "_act_tables_patched", False):
        return
    orig = hw_specs.get_activation_tables

    @functools.cache
    def only_combined(arch):
        return {
            name: (funcs if name == "natural_log_exp_and_others" else set())
            for name, funcs in orig(arch).items()
        }

    bacc.get_activation_tables = only_combined
    bacc._act_tables_patched = True


def build_bass(m_total: int = M, ch: int = CH) -> bass.Bass:
    _patch_act_tables()
    nch = m_total // ch
    nspc = ch // SUB
    nsub = m_total // SUB

    nc = bacc.Bacc(trn_type="TRN2", target_bir_lowering=False, debug=False)

    et2_d = nc.dram_tensor("et2", [D, NSH], BF16, kind="ExternalInput").ap()
    nsq_d = nc.dram_tensor("negsqe", [NSH, 1], F32, kind="ExternalInput").ap()
    refwt_d = nc.dram_tensor("refwt", [D, m_total], BF16, kind="ExternalInput").ap()
    w_d = nc.dram_tensor("wmask", [NSH, m_total], mybir.dt.float16, kind="ExternalInput").ap()
    out_d = nc.dram_tensor("out", [NSH, m_total], F32, kind="ExternalOutput").ap()

    with tile.TileContext(nc) as tc, ExitStack() as ctx:
        const = ctx.enter_context(tc.tile_pool(name="const", bufs=1))
        bigp = ctx.enter_context(tc.tile_pool(name="emaskp", bufs=1))
        rtp = ctx.enter_context(tc.tile_pool(name="rt", bufs=4))
        wp = ctx.enter_context(tc.tile_pool(name="wp", bufs=4))
        ep = ctx.enter_context(tc.tile_pool(name="ep", bufs=4))
        psp = ctx.enter_context(tc.tile_pool(name="ps", bufs=4, space="PSUM"))

        zbias = const.tile([NSH, 1], F32)
        nc.vector.memset(zbias, 0.0)
        et2 = const.tile([D, NSH], BF16)
        nc.sync.dma_start(out=et2, in_=et2_d)
        nsq = const.tile([NSH, 1], F32)
        nc.sync.dma_start(out=nsq, in_=nsq_d)

        zparts = const.tile([NSH, m_total // (2 * SUB)], F32)
        emask = bigp.tile([NSH, m_total], F32)

        PAIR = 2 * SUB  # exp/mult run at 1024 wide over a 2-bank psum tile
        for i in range(nch):
            csl = slice(i * ch, (i + 1) * ch)
            rt = rtp.tile([D, ch], BF16)
            nc.sync.dma_start(out=rt, in_=refwt_d[:, csl])
            w = wp.tile([NSH, ch], mybir.dt.float16)
            nc.sync.dma_start(out=w, in_=w_d[:, csl])

            for s in range(ch // PAIR):
                psl = slice(s * PAIR, (s + 1) * PAIR)
                k = i * (ch // PAIR) + s
                ksl = slice(k * PAIR, (k + 1) * PAIR)
                ps = psp.tile([NSH, PAIR], F32)
                for h in range(2):
                    nc.tensor.matmul(
                        ps[:, h * SUB:(h + 1) * SUB], lhsT=et2,
                        rhs=rt[:, s * PAIR + h * SUB: s * PAIR + (h + 1) * SUB],
                        start=True, stop=True,
                    )
                ex = ep.tile([NSH, PAIR], F32)
                # e' = exp(2*cross - sq_e)
                nc.scalar.activation(
                    out=ex, in_=ps,
                    func=mybir.ActivationFunctionType.Exp,
                    bias=nsq, scale=1.0,
                )
                # emask = e' * W  (0 at masked), Z-partial = fused row-sum
                nc.vector.scalar_tensor_tensor(
                    out=emask[:, ksl], in0=ex, scalar=1.0, in1=w[:, psl],
                    op0=mybir.AluOpType.mult, op1=mybir.AluOpType.mult,
                    accum_out=zparts[:, k:k + 1],
                )


        zsum = const.tile([NSH, 1], F32)
        nc.vector.tensor_reduce(
            zsum, zparts, axis=mybir.AxisListType.X, op=mybir.AluOpType.add
        )
        rz = const.tile([NSH, 1], F32)
        nc.vector.reciprocal(rz, zsum)
        neg_lnz = const.tile([NSH, 1], F32)
        # ln(1/Z) = -lnZ
        nc.scalar.activation(
            out=neg_lnz, in_=rz, func=mybir.ActivationFunctionType.Ln,
            bias=zbias, scale=1.0,
        )

        for i in range(nch):
            sl = slice(i * ch, (i + 1) * ch)
            # out = ln(e_masked / Z); ln(0) = -inf at masked
            nc.scalar.activation(
                out=emask[:, sl], in_=emask[:, sl],
                func=mybir.ActivationFunctionType.Ln,
                bias=zbias, scale=rz,
            )
            nc.sync.dma_start(out=out_d[:, sl], in_=emask[:, sl])

    nc.compile()
    return nc


def make_in_maps(embeddings: np.ndarray, ref_weight: np.ndarray, inputs: np.ndarray):
    import ml_dtypes

    embeddings = np.ascontiguousarray(np.asarray(embeddings, dtype=np.float32))
    ref_weight = np.asarray(ref_weight, dtype=np.float32)
    inputs = np.asarray(inputs)
    refwt = np.ascontiguousarray(ref_weight.T).astype(ml_dtypes.bfloat16)  # [D, M]
    sqr = (ref_weight * ref_weight).sum(axis=1)[None, :]                   # [1, M]
    wmask = np.where(inputs == 0, np.exp(-sqr), np.float32(0.0)).astype(np.float16)
    in_maps = []
    for c in range(NCORES):
        e = embeddings[c * NSH:(c + 1) * NSH]
        in_maps.append({
            "et2": (np.ascontiguousarray(e.T) * np.float32(2.0)).astype(ml_dtypes.bfloat16),
            "negsqe": -(e * e).sum(axis=1, keepdims=True).astype(np.float32),
            "refwt": refwt,
            "wmask": np.ascontiguousarray(wmask[c * NSH:(c + 1) * NSH]),
        })
    return in_maps


_NC_CACHE: dict = {}


def get_nc() -> bass.Bass:
    if "nc" not in _NC_CACHE:
        _NC_CACHE["nc"] = build_bass()
    return _NC_CACHE["nc"]


def kernel(embeddings: np.ndarray, ref_weight: np.ndarray, inputs: np.ndarray,
           **_ignored) -> np.ndarray:
    nc = get_nc()
    in_maps = make_in_maps(embeddings, ref_weight, inputs)
    res = run_bass_kernel_spmd(nc, in_maps, list(range(NCORES)))
    out = np.concatenate([res.results[c]["out"] for c in range(NCORES)], axis=0)
    return out
